# revision 7
# baseline (speedup 1.0000x reference)
"""Depthwise causal conv1d kernel for Trainium2 (8 NeuronCores, SPMD).

Problem: x [B=8, T=4096, C=512] f32, weight [C=512, K=4] f32.
out[b, t, c] = sum_k weight[c, k] * x[b, t - 3 + k, c]   (causal, zero-pad)

Strategy (v5):
  - Data-parallel over batch: core b handles x[b].
  - Host-side layout: channels-first x[b].T padded with 3 leading zeros
    -> 4 chunk tiles [128, 4099] fp16; fp16 output (host upcasts).
    Diagonal stationary weight tiles built on the host and DMA'd.
  - Inputs stream on TWO queues in parallel: SP carries the PE chunks
    (2,3) + diag weights, ACT's HWDGE queue carries the elementwise
    chunks (0,1) + scalar weights, so PE and DVE both start ~as early
    as the post-init DMA fence allows.
  - PE: chunks 2,3 (4 accumulating diag-matmuls per 512 PSUM slice).
  - ACT: tap-0 products for chunks 0,1 (activation, per-partition
    scale), PSUM->fp16 copies for chunks 2,3 (final copy staged finely
    to shorten the tail), ships chunks 2,3.
  - DVE: taps 1-3 products (tensor_scalar, 4x fp16 @2048 wide) + most
    combining adds (tensor_tensor, 2x fp16).
  - GpSimd: first-half a23 adds (slow but off the tail), ships chunk 0.
  - SP ships chunk 1 after its input stream drains.
  - exec_time ~= last_work_end + ~5us fixed (framework teardown), so
    few/wide ops and a short tail matter more than engine balance.
"""

import numpy as np

B, T, C, K = 8, 4096, 512, 4
P = 128  # partitions
NCHUNK = C // P  # 4 channel chunks
TP = T + K - 1  # padded time = 4099
NW = NCHUNK * K  # 16 weight columns
H = 2048  # op width
HSPLIT = H + K  # 2052: first-half input split (halo incl.)
E0, E1 = 0, 1  # elementwise chunks
PE_CHUNKS = (2, 3)

_compiled = None


def _build():
    import concourse.bacc as bacc
    import concourse.mybir as mybir
    from concourse.tile import TileContext

    f32 = mybir.dt.float32
    f16 = mybir.dt.float16
    Alu = mybir.AluOpType
    nc = bacc.Bacc(enable_partition_id=False)

    wtf32_d = nc.declare_dram_parameter("wtf32", [P, NW], f32, isOutput=False)
    wd_d = {
        c: nc.declare_dram_parameter(f"wd{c}", [P, K * P], f16, isOutput=False)
        for c in PE_CHUNKS
    }
    xw_d = nc.declare_dram_parameter("xw", [P, NCHUNK * TP], f16, isOutput=False)
    out_d = nc.declare_dram_parameter("out", [C, T], f16, isOutput=True)

    with TileContext(nc) as tc:
        with (
            tc.tile_pool(name="xpool", bufs=1) as xpool,
            tc.tile_pool(name="wpool", bufs=1) as wpool,
            tc.tile_pool(name="tpool", bufs=1) as tpool,
            tc.tile_pool(name="opool", bufs=1) as opool,
            tc.tile_pool(name="ppool", bufs=2, space="PSUM") as ppool,
        ):
            c2, c3 = PE_CHUNKS

            wtf32 = wpool.tile([P, NW], f32, name="wtf32", tag="wtf32")
            wd = {
                c: wpool.tile([P, K * P], f16, name=f"wd{c}", tag=f"wd{c}")
                for c in PE_CHUNKS
            }
            xt = {
                c: xpool.tile([P, TP], f16, name=f"xt{c}", tag=f"xt{c}")
                for c in range(NCHUNK)
            }

            def load_x(eng, c, lo, hi):
                eng.dma_start(
                    out=xt[c][:, lo:hi], in_=xw_d[:, c * TP + lo : c * TP + hi]
                )

            # ACT queue: scalar weights + elementwise chunks
            nc.scalar.dma_start(out=wtf32, in_=wtf32_d[:, :])
            load_x(nc.scalar, E0, 0, HSPLIT)
            load_x(nc.scalar, E0, HSPLIT, TP)
            load_x(nc.scalar, E1, 0, HSPLIT)
            load_x(nc.scalar, E1, HSPLIT, TP)
            # SP queue: diag weights + PE chunks
            nc.sync.dma_start(out=wd[c2], in_=wd_d[c2][:, :])
            load_x(nc.sync, c2, 0, HSPLIT)
            load_x(nc.sync, c2, HSPLIT, TP)
            nc.sync.dma_start(out=wd[c3], in_=wd_d[c3][:, :])
            load_x(nc.sync, c3, 0, HSPLIT)
            load_x(nc.sync, c3, HSPLIT, TP)

            ot = {
                c: opool.tile([P, T], f16, name=f"ot{c}", tag=f"ot{c}")
                for c in range(NCHUNK)
            }

            pts = {}

            def pe_chunk(c):
                for half in range(2):
                    pt = ppool.tile([P, H], f32, name="pt", tag="pt")
                    pts[(c, half)] = pt
                    for s in range(4):
                        base = half * H + s * 512
                        for k in range(K):
                            nc.tensor.matmul(
                                pt[:, s * 512 : (s + 1) * 512],
                                wd[c][:, k * P : (k + 1) * P],
                                xt[c][:, base + k : base + k + 512],
                                start=(k == 0),
                                stop=(k == K - 1),
                            )

            def act_cp(c, half, lo, width):
                nc.scalar.copy(
                    ot[c][:, half * H + lo : half * H + lo + width],
                    pts[(c, half)][:, lo : lo + width],
                )

            def ship(eng, c, lo, width):
                eng.dma_start(
                    out=out_d[c * P : (c + 1) * P, lo : lo + width],
                    in_=ot[c][:, lo : lo + width],
                )

            # elementwise temps (per chunk)
            tiles = {}
            for nm in ("m0", "m1", "m2", "m3", "a01", "a23"):
                for c in (E0, E1):
                    tiles[(nm, c)] = tpool.tile(
                        [P, T], f16, name=f"{nm}_{c}", tag=f"{nm}_{c}"
                    )

            def wsl(c, k):
                return wtf32[:, c * K + k : c * K + k + 1]

            def hsl(h):
                return slice(h * H, (h + 1) * H)

            def act_m0(c, h):
                nc.scalar.activation(
                    out=tiles[("m0", c)][:, hsl(h)],
                    in_=xt[c][:, h * H : h * H + H],
                    func=mybir.ActivationFunctionType.Copy,
                    scale=wsl(c, 0),
                )

            def mult(c, tap, h):
                nc.vector.tensor_scalar(
                    out=tiles[(f"m{tap}", c)][:, hsl(h)],
                    in0=xt[c][:, h * H + tap : h * H + tap + H],
                    scalar1=wsl(c, tap),
                    scalar2=None,
                    op0=Alu.mult,
                )

            def add(c, dst, x, y, h, eng=None):
                (eng or nc.vector).tensor_tensor(
                    out=(ot[c] if dst == "ot" else tiles[(dst, c)])[:, hsl(h)],
                    in0=tiles[(x, c)][:, hsl(h)],
                    in1=tiles[(y, c)][:, hsl(h)],
                    op=Alu.add,
                )

            # ---- emission in dataflow order ----
            pe_chunk(c2)

            act_m0(E0, 0)
            act_m0(E0, 1)
            # DVE chunk 0, first half mults (m2/m3 first so GpSimd starts)
            mult(E0, 2, 0)
            mult(E0, 3, 0)
            mult(E0, 1, 0)
            add(E0, "a23", "m2", "m3", 0, eng=nc.gpsimd)
            add(E0, "a01", "m0", "m1", 0)
            mult(E0, 2, 1)
            mult(E0, 3, 1)
            mult(E0, 1, 1)
            add(E0, "a01", "m0", "m1", 1)
            add(E0, "a23", "m2", "m3", 1)
            add(E0, "ot", "a01", "a23", 0)
            add(E0, "ot", "a01", "a23", 1)

            act_m0(E1, 0)
            act_m0(E1, 1)
            # DVE chunk 1
            mult(E1, 2, 0)
            mult(E1, 3, 0)
            mult(E1, 1, 0)
            add(E1, "a23", "m2", "m3", 0, eng=nc.gpsimd)
            add(E1, "a01", "m0", "m1", 0)
            mult(E1, 2, 1)
            mult(E1, 3, 1)
            mult(E1, 1, 1)
            add(E1, "a01", "m0", "m1", 1)
            add(E1, "a23", "m2", "m3", 1)
            add(E1, "ot", "a01", "a23", 0)
            add(E1, "ot", "a01", "a23", 1)

            # GpSimd ships chunk 0 from its own (SWDGE) queue
            ship(nc.gpsimd, E0, 0, T)

            # ACT: chunk-2 copies + ships (emitted after PE c2)
            act_cp(c2, 0, 0, H)
            ship(nc.scalar, c2, 0, H)
            act_cp(c2, 1, 0, H)
            ship(nc.scalar, c2, H, H)

            pe_chunk(c3)
            act_cp(c3, 0, 0, H)
            ship(nc.scalar, c3, 0, H)
            # last PE tile staged finely to shorten the tail
            act_cp(c3, 1, 0, 1024)
            ship(nc.scalar, c3, H, 1024)
            act_cp(c3, 1, 1024, 512)
            act_cp(c3, 1, 1536, 512)
            ship(nc.scalar, c3, H + 1024, 1024)

            # SP queue is idle after inputs: ship chunk 1
            ship(nc.sync, E1, 0, H)
            ship(nc.sync, E1, H, H)

    nc.compile()
    return nc


def _prep_inputs(x: np.ndarray, weight: np.ndarray):
    # wcol[p, chunk*K + k] = weight[chunk*P + p, k]
    wcol = np.ascontiguousarray(
        weight.reshape(NCHUNK, P, K).transpose(1, 0, 2).reshape(P, NW)
    )
    wtf32 = wcol.astype(np.float32)
    wcol16 = wcol.astype(np.float16)
    wds = {}
    rng = np.arange(P)
    for c in PE_CHUNKS:
        wdm = np.zeros((P, K * P), dtype=np.float16)
        for k in range(K):
            wdm[rng, k * P + rng] = wcol16[:, c * K + k]
        wds[c] = wdm
    xs = []
    for b in range(B):
        xp = np.zeros((C, TP), dtype=np.float32)
        xp[:, K - 1 :] = x[b].T  # [512, 4099], 3 leading zeros
        xw = np.ascontiguousarray(
            xp.reshape(NCHUNK, P, TP).transpose(1, 0, 2).reshape(P, NCHUNK * TP)
        ).astype(np.float16)
        xs.append(xw)
    return xs, wtf32, wds


def _ensure_axon_hooks():
    """This image's antenv package lacks axon_hooks; synthesize it so a
    trace=True / BASS_TRACE run of run_bass_kernel_spmd can profile
    instead of crashing on import."""
    import sys
    import types

    if "antenv.axon_hooks" in sys.modules:
        return
    mod = types.ModuleType("antenv.axon_hooks")
    state = {"hook": None}
    mod.set_axon_ntff_profile_hook = lambda h: state.__setitem__("hook", h)
    mod.get_axon_ntff_profile_hook = lambda: state["hook"]
    sys.modules["antenv.axon_hooks"] = mod
    try:
        if "/root/.axon_site" not in sys.path:
            sys.path.insert(0, "/root/.axon_site")
        from trn_agent_boot.trn_boot import _ntff_profile_via_ctypes

        mod.set_axon_ntff_profile_hook(
            _ntff_profile_via_ctypes("/opt/axon/libaxon_pjrt.so")
        )
    except Exception:
        pass  # hook stays None; concourse degrades to no-trace


def _in_maps(x, weight):
    xs, wtf32, wds = _prep_inputs(x, weight)
    return [
        {
            "xw": xs[b],
            "wtf32": wtf32,
            **{f"wd{c}": wds[c] for c in PE_CHUNKS},
        }
        for b in range(B)
    ]


def kernel(x: np.ndarray, weight: np.ndarray) -> np.ndarray:
    global _compiled
    _ensure_axon_hooks()
    from concourse import bass_utils

    x = np.ascontiguousarray(x, dtype=np.float32)
    weight = np.ascontiguousarray(weight, dtype=np.float32)

    if _compiled is None:
        _compiled = _build()
    nc = _compiled

    res = bass_utils.run_bass_kernel_spmd(
        nc, _in_maps(x, weight), core_ids=list(range(B))
    )

    out = np.empty((B, T, C), dtype=np.float32)
    for b in range(B):
        out[b] = np.asarray(res.results[b]["out"]).astype(np.float32).T
    return out


# revision 9
# speedup vs baseline: 1.0526x; 1.0526x over previous
"""Depthwise causal conv1d kernel for Trainium2 (8 NeuronCores, SPMD).

Problem: x [B=8, T=4096, C=512] f32, weight [C=512, K=4] f32.
out[b, t, c] = sum_k weight[c, k] * x[b, t - 3 + k, c]   (causal, zero-pad)

Strategy (v6):
  - Data-parallel over batch: core b handles x[b].
  - Host-side layout: channels-first x[b].T padded with 3 leading zeros,
    fp16, split per chunk into TWO overlapping half tiles (cols 0..2051
    and 2048..4098).  Separate tiles matter: dependency tracking is
    tile-granular, so a single [128,4099] tile would make the first
    matmul wait for the chunk's *second* DMA as well.
  - PE computes chunks 2,3 (4 accumulating diag-matmuls per 512 PSUM
    slice; host-built diagonal stationary tiles).
  - Chunks 0,1 are elementwise at 2048 width: ACT does tap-0 products
    (activation, per-partition scale), DVE taps 1-3 (tensor_scalar, 4x
    fp16) + combining adds (tensor_tensor, 2x), GpSimd one early a23.
  - PSUM->fp16 copies: ACT (chunk 2 + staged final tile of chunk 3),
    GpSimd (chunk 3 first tile).  Ships spread over SP/GpSimd queues.
  - Inputs stream on two queues (SP + ACT); the whole problem is
    chip-HBM-bound (~69MB for 8 cores), so the goal is to keep both
    directions streaming from the earliest fence and keep every
    engine's serial work under the stream time.
"""

import numpy as np

B, T, C, K = 8, 4096, 512, 4
P = 128  # partitions
NCHUNK = C // P  # 4 channel chunks
TP = T + K - 1  # padded time = 4099
NW = NCHUNK * K  # 16 weight columns
H = 2048  # op width / half size
HA = H + K  # 2052 cols in first half tile (incl halo)
HB = TP - H  # 2051 cols in second half tile (starts at col 2048)
E0, E1 = 0, 1  # elementwise chunks
PE_CHUNKS = (2, 3)

_compiled = None


def _build():
    import concourse.bacc as bacc
    import concourse.mybir as mybir
    from concourse.tile import TileContext

    f32 = mybir.dt.float32
    f16 = mybir.dt.float16
    Alu = mybir.AluOpType
    nc = bacc.Bacc(enable_partition_id=False)

    wtf32_d = nc.declare_dram_parameter("wtf32", [P, NW], f32, isOutput=False)
    wd_d = {
        c: nc.declare_dram_parameter(f"wd{c}", [P, K * P], f16, isOutput=False)
        for c in PE_CHUNKS
    }
    xw_d = nc.declare_dram_parameter("xw", [P, NCHUNK * TP], f16, isOutput=False)
    out_d = nc.declare_dram_parameter("out", [C, T], f16, isOutput=True)

    with TileContext(nc) as tc:
        with (
            tc.tile_pool(name="xpool", bufs=1) as xpool,
            tc.tile_pool(name="wpool", bufs=1) as wpool,
            tc.tile_pool(name="tpool", bufs=1) as tpool,
            tc.tile_pool(name="opool", bufs=1) as opool,
            tc.tile_pool(name="ppool", bufs=2, space="PSUM") as ppool,
        ):
            c2, c3 = PE_CHUNKS

            wtf32 = wpool.tile([P, NW], f32, name="wtf32", tag="wtf32")
            wd = {
                c: wpool.tile([P, K * P], f16, name=f"wd{c}", tag=f"wd{c}")
                for c in PE_CHUNKS
            }
            # per chunk: two overlapping half tiles
            xa = {
                c: xpool.tile([P, HA], f16, name=f"xa{c}", tag=f"xa{c}")
                for c in range(NCHUNK)
            }
            xb = {
                c: xpool.tile([P, HB], f16, name=f"xb{c}", tag=f"xb{c}")
                for c in range(NCHUNK)
            }

            def load_half(eng, c, half):
                if half == 0:
                    eng.dma_start(
                        out=xa[c], in_=xw_d[:, c * TP : c * TP + HA]
                    )
                else:
                    eng.dma_start(
                        out=xb[c], in_=xw_d[:, c * TP + H : (c + 1) * TP]
                    )

            # ACT queue: scalar weights + chunk-0 halves
            nc.scalar.dma_start(out=wtf32, in_=wtf32_d[:, :])
            load_half(nc.scalar, E0, 0)
            load_half(nc.scalar, E0, 1)
            # SP queue: diag weights + PE chunks + chunk-1 halves
            nc.sync.dma_start(out=wd[c2], in_=wd_d[c2][:, :])
            load_half(nc.sync, c2, 0)
            load_half(nc.sync, c2, 1)
            nc.sync.dma_start(out=wd[c3], in_=wd_d[c3][:, :])
            load_half(nc.sync, c3, 0)
            load_half(nc.sync, c3, 1)
            load_half(nc.sync, E1, 0)
            load_half(nc.sync, E1, 1)

            ot = {
                c: opool.tile([P, T], f16, name=f"ot{c}", tag=f"ot{c}")
                for c in range(NCHUNK)
            }

            pts = {}

            def pe_chunk(c):
                for half in range(2):
                    src = xa[c] if half == 0 else xb[c]
                    pt = ppool.tile([P, H], f32, name="pt", tag="pt")
                    pts[(c, half)] = pt
                    for s in range(4):
                        for k in range(K):
                            nc.tensor.matmul(
                                pt[:, s * 512 : (s + 1) * 512],
                                wd[c][:, k * P : (k + 1) * P],
                                src[:, s * 512 + k : s * 512 + k + 512],
                                start=(k == 0),
                                stop=(k == K - 1),
                            )

            def ship(eng, c, lo, width):
                eng.dma_start(
                    out=out_d[c * P : (c + 1) * P, lo : lo + width],
                    in_=ot[c][:, lo : lo + width],
                )

            # elementwise temps (per chunk)
            tiles = {}
            for nm in ("m0", "m1", "m2", "m3", "a01", "a23"):
                for c in (E0, E1):
                    tiles[(nm, c)] = tpool.tile(
                        [P, T], f16, name=f"{nm}_{c}", tag=f"{nm}_{c}"
                    )

            def wsl(c, k):
                return wtf32[:, c * K + k : c * K + k + 1]

            def hsl(h):
                return slice(h * H, (h + 1) * H)

            def act_m0(c, h):
                nc.scalar.activation(
                    out=tiles[("m0", c)][:, hsl(h)],
                    in_=(xa[c] if h == 0 else xb[c])[:, 0:H],
                    func=mybir.ActivationFunctionType.Copy,
                    scale=wsl(c, 0),
                )

            def mult(c, tap, h):
                nc.vector.tensor_scalar(
                    out=tiles[(f"m{tap}", c)][:, hsl(h)],
                    in0=(xa[c] if h == 0 else xb[c])[:, tap : tap + H],
                    scalar1=wsl(c, tap),
                    scalar2=None,
                    op0=Alu.mult,
                )

            def add(c, dst, x, y, h, eng=None):
                (eng or nc.vector).tensor_tensor(
                    out=(ot[c] if dst == "ot" else tiles[(dst, c)])[:, hsl(h)],
                    in0=tiles[(x, c)][:, hsl(h)],
                    in1=tiles[(y, c)][:, hsl(h)],
                    op=Alu.add,
                )

            # ---- emission in dataflow order ----
            pe_chunk(c2)

            act_m0(E0, 0)
            act_m0(E0, 1)
            # DVE chunk 0 (m2/m3 first so GpSimd's a23a can start)
            mult(E0, 2, 0)
            mult(E0, 3, 0)
            mult(E0, 1, 0)
            add(E0, "a23", "m2", "m3", 0, eng=nc.gpsimd)
            add(E0, "a01", "m0", "m1", 0)
            mult(E0, 2, 1)
            mult(E0, 3, 1)
            mult(E0, 1, 1)
            add(E0, "a01", "m0", "m1", 1)
            add(E0, "a23", "m2", "m3", 1)
            add(E0, "ot", "a01", "a23", 0)
            add(E0, "ot", "a01", "a23", 1)

            # ACT: chunk-2 copies; SP ships them
            nc.scalar.copy(ot[c2][:, hsl(0)], pts[(c2, 0)])
            ship(nc.sync, c2, 0, H)
            nc.scalar.copy(ot[c2][:, hsl(1)], pts[(c2, 1)])
            ship(nc.sync, c2, H, H)

            act_m0(E1, 0)
            act_m0(E1, 1)
            # DVE chunk 1 (owns both a23 halves)
            mult(E1, 2, 0)
            mult(E1, 3, 0)
            mult(E1, 1, 0)
            add(E1, "a23", "m2", "m3", 0)
            add(E1, "a01", "m0", "m1", 0)
            mult(E1, 2, 1)
            mult(E1, 3, 1)
            mult(E1, 1, 1)
            add(E1, "a01", "m0", "m1", 1)
            add(E1, "a23", "m2", "m3", 1)
            add(E1, "ot", "a01", "a23", 0)
            add(E1, "ot", "a01", "a23", 1)

            # GpSimd: ship chunk 0 from its own (SWDGE) queue
            ship(nc.gpsimd, E0, 0, T)

            pe_chunk(c3)
            # chunk-3 first tile: ACT copy, SP ship
            nc.scalar.copy(ot[c3][:, hsl(0)], pts[(c3, 0)])
            ship(nc.sync, c3, 0, H)
            # chunk-3 last tile: ACT staged finely, SP ships
            nc.scalar.copy(ot[c3][:, H : H + 1024], pts[(c3, 1)][:, 0:1024])
            ship(nc.sync, c3, H, 1024)
            nc.scalar.copy(
                ot[c3][:, H + 1024 : H + 1536], pts[(c3, 1)][:, 1024:1536]
            )
            nc.scalar.copy(
                ot[c3][:, H + 1536 : H + 2048], pts[(c3, 1)][:, 1536:2048]
            )
            ship(nc.sync, c3, H + 1024, 1024)

            # SP ships chunk 1 as its outs land
            ship(nc.sync, E1, 0, H)
            ship(nc.sync, E1, H, H)

    nc.compile()
    return nc


def _prep_inputs(x: np.ndarray, weight: np.ndarray):
    # wcol[p, chunk*K + k] = weight[chunk*P + p, k]
    wcol = np.ascontiguousarray(
        weight.reshape(NCHUNK, P, K).transpose(1, 0, 2).reshape(P, NW)
    )
    wtf32 = wcol.astype(np.float32)
    wcol16 = wcol.astype(np.float16)
    wds = {}
    rng = np.arange(P)
    for c in PE_CHUNKS:
        wdm = np.zeros((P, K * P), dtype=np.float16)
        for k in range(K):
            wdm[rng, k * P + rng] = wcol16[:, c * K + k]
        wds[c] = wdm
    xs = []
    for b in range(B):
        xp = np.zeros((C, TP), dtype=np.float32)
        xp[:, K - 1 :] = x[b].T  # [512, 4099], 3 leading zeros
        xw = np.ascontiguousarray(
            xp.reshape(NCHUNK, P, TP).transpose(1, 0, 2).reshape(P, NCHUNK * TP)
        ).astype(np.float16)
        xs.append(xw)
    return xs, wtf32, wds


def _ensure_axon_hooks():
    """This image's antenv package lacks axon_hooks; synthesize it so a
    trace=True / BASS_TRACE run of run_bass_kernel_spmd can profile
    instead of crashing on import."""
    import sys
    import types

    if "antenv.axon_hooks" in sys.modules:
        return
    mod = types.ModuleType("antenv.axon_hooks")
    state = {"hook": None}
    mod.set_axon_ntff_profile_hook = lambda h: state.__setitem__("hook", h)
    mod.get_axon_ntff_profile_hook = lambda: state["hook"]
    sys.modules["antenv.axon_hooks"] = mod
    try:
        if "/root/.axon_site" not in sys.path:
            sys.path.insert(0, "/root/.axon_site")
        from trn_agent_boot.trn_boot import _ntff_profile_via_ctypes

        mod.set_axon_ntff_profile_hook(
            _ntff_profile_via_ctypes("/opt/axon/libaxon_pjrt.so")
        )
    except Exception:
        pass  # hook stays None; concourse degrades to no-trace


def _in_maps(x, weight):
    xs, wtf32, wds = _prep_inputs(x, weight)
    return [
        {
            "xw": xs[b],
            "wtf32": wtf32,
            **{f"wd{c}": wds[c] for c in PE_CHUNKS},
        }
        for b in range(B)
    ]


def kernel(x: np.ndarray, weight: np.ndarray) -> np.ndarray:
    global _compiled
    _ensure_axon_hooks()
    from concourse import bass_utils

    x = np.ascontiguousarray(x, dtype=np.float32)
    weight = np.ascontiguousarray(weight, dtype=np.float32)

    if _compiled is None:
        _compiled = _build()
    nc = _compiled

    res = bass_utils.run_bass_kernel_spmd(
        nc, _in_maps(x, weight), core_ids=list(range(B))
    )

    out = np.empty((B, T, C), dtype=np.float32)
    for b in range(B):
        out[b] = np.asarray(res.results[b]["out"]).astype(np.float32).T
    return out
